# revision 1
# baseline (speedup 1.0000x reference)
"""Trainium2 Bass kernel for nn_BatchedTeacherPolicy.

2048 independent per-teacher MLPs (obs-norm -> 48->512->256->128->12,
ELU between layers, tanh at the end). Pure data parallel: 256 teachers
per NeuronCore across 8 cores.

Layout: teacher-on-partition. Each SBUF partition holds one teacher's
weights/activations; the per-teacher matvec y[o] = b[o] + sum_i W[o,i]x[i]
is one fused DVE tensor_tensor_reduce per output neuron o, computed for
128 teachers (partitions) simultaneously. Weight DMAs are fully
contiguous per partition (W[n, o0:o1, :] blocks).
"""

from contextlib import ExitStack

import numpy as np

import concourse.bass as bass
import concourse.bacc as bacc
import concourse.tile as tile
from concourse import mybir
from concourse.bass_utils import run_bass_kernel_spmd

N, OBS = 2048, 48
DIMS = [(512, 48), (256, 512), (128, 256), (12, 128)]  # (out, in) per layer
N_CORES = 8
NPC = N // N_CORES  # teachers per core
P = 128             # partitions = teachers per group
G = NPC // P        # groups per core
# o-chunk per layer: sized so W DMA chunks are ~2-4 MB
OCHUNK = [128, 16, 32, 12]

F32 = mybir.dt.float32
AF = mybir.ActivationFunctionType
ALU = mybir.AluOpType

# Layer 1 output split: o < L1_DVE computed by DVE fused multiply-reduce;
# the last L1_PE columns computed on TensorE from a host-transposed W1
# slice (keeps DVE, the bottleneck engine, under the DMA roofline).
USE_PE = False
L1_PE = 64 if USE_PE else 0
L1_DVE = DIMS[1][0] - L1_PE
L1_CI = DIMS[1][1] // P  # 4 contraction chunks of 128

_cached = {}


def _build_bass():
    nc = bacc.Bacc(trn_type="TRN2", target_bir_lowering=False)

    obs_d = nc.dram_tensor("obs", [NPC, OBS], F32, kind="ExternalInput")
    mean_d = nc.dram_tensor("mean", [NPC, OBS], F32, kind="ExternalInput")
    std_d = nc.dram_tensor("std", [NPC, OBS], F32, kind="ExternalInput")
    W_d, b_d = [], []
    for li, (o, i) in enumerate(DIMS):
        o_dve = L1_DVE if li == 1 else o
        W_d.append(
            nc.dram_tensor(f"W{li}", [NPC, o_dve, i], F32, kind="ExternalInput")
        )
        b_d.append(nc.dram_tensor(f"b{li}", [NPC, o], F32, kind="ExternalInput"))
    # host-transposed slice of W1: [g, ci, i_local(part), teacher, o]
    w1t_d = None
    if USE_PE:
        w1t_d = nc.dram_tensor(
            "W1T", [G, L1_CI, P, P, L1_PE], F32, kind="ExternalInput"
        )
    out_d = nc.dram_tensor("out", [NPC, DIMS[-1][0]], F32, kind="ExternalOutput")

    from concourse.masks import make_identity

    with ExitStack() as ctx:
        tc = ctx.enter_context(tile.TileContext(nc))
        wpool = ctx.enter_context(tc.tile_pool(name="wpool", bufs=5))
        xpool = ctx.enter_context(tc.tile_pool(name="xpool", bufs=3))
        spool = ctx.enter_context(tc.tile_pool(name="spool", bufs=2))
        bpool = ctx.enter_context(tc.tile_pool(name="bpool", bufs=2))
        ppool = ctx.enter_context(tc.tile_pool(name="ppool", bufs=2, space="PSUM"))
        ipool = ctx.enter_context(tc.tile_pool(name="ipool", bufs=1))

        ident = ipool.tile([P, P], F32)
        make_identity(nc, ident)

        def emit_norm(g):
            n0 = g * P

            # ---- obs normalization: x0 = clip((obs - mean)/std, -5, 5) ----
            obs_t = spool.tile([P, OBS], F32, tag="nrm")
            nc.sync.dma_start(out=obs_t, in_=obs_d[n0 : n0 + P, :])
            mean_t = spool.tile([P, OBS], F32, tag="nrm")
            nc.sync.dma_start(out=mean_t, in_=mean_d[n0 : n0 + P, :])
            std_t = spool.tile([P, OBS], F32, tag="nrm")
            nc.sync.dma_start(out=std_t, in_=std_d[n0 : n0 + P, :])

            # Each DVE op may carry at most ONE new semaphore wait (TRN2
            # TT-struct limit), so feed multi-operand ops through
            # single-input ops that absorb the DMA waits first.
            nmean = spool.tile([P, OBS], F32, tag="nmean")
            nc.vector.tensor_scalar_mul(nmean, mean_t, -1.0)
            rstd = spool.tile([P, OBS], F32, tag="rstd")
            nc.vector.reciprocal(rstd, std_t)
            x = xpool.tile([P, OBS], F32, tag="x", name=f"x_in_{g}")
            nc.vector.tensor_add(x, obs_t, nmean)
            nc.vector.tensor_mul(x, x, rstd)
            nc.vector.tensor_scalar(
                out=x, in0=x, scalar1=-5.0, scalar2=5.0,
                op0=ALU.max, op1=ALU.min,
            )
            return x

        def emit_layer(g, li, x):
            n0 = g * P
            O, I = DIMS[li]
            if True:
                bt = bpool.tile([P, O], F32, tag="bias", name=f"b_{g}_{li}")
                nc.sync.dma_start(out=bt, in_=b_d[li][n0 : n0 + P, :])
                y = xpool.tile([P, O], F32, tag="y", name=f"y_{g}_{li}")
                o_dve = L1_DVE if li == 1 else O

                if li == 1 and USE_PE:
                    # TensorE path for y[:, L1_DVE:]: x1 transposed via PE,
                    # then per-teacher matvecs with the host-transposed W1
                    # slice as the stationary operand. ci-outer order keeps
                    # exactly one W1T tile live at a time; each PSUM column
                    # t accumulates across the four ci passes.
                    x1t = xpool.tile([P, L1_CI, P], F32, tag="x1t", name=f"x1t_{g}")
                    for ci in range(L1_CI):
                        pst = ppool.tile([P, P], F32, tag="pst", name=f"pst_{g}_{ci}")
                        nc.tensor.transpose(
                            pst, x[:, ci * P : (ci + 1) * P], ident
                        )
                        nc.scalar.copy(x1t[:, ci, :], pst)
                    yps = ppool.tile([L1_PE, P], F32, tag="yps", name=f"yps_{g}")
                    TH = 32  # teachers per W1T DMA tile
                    for th0 in range(0, P, TH):
                        wtts = []
                        for ci in range(L1_CI):
                            wtt = wpool.tile(
                                [P, TH, L1_PE], F32, tag="w1t", bufs=6,
                                name=f"w1t_{g}_{th0}_{ci}",
                            )
                            # ACT HWDGE ring: these DMAs wait on PE slot
                            # reuse and would stall the SP ring's weight
                            # stream (HWDGE is FIFO per issuing engine).
                            # Emission order guarantees every ELU Exp that
                            # gates DVE progress precedes them in the ACT
                            # stream.
                            nc.scalar.dma_start(
                                out=wtt, in_=w1t_d[g, ci, :, th0 : th0 + TH, :]
                            )
                            wtts.append(wtt)
                        # t-outer, ci-inner: each PSUM column's accumulation
                        # group runs start..stop contiguously (interleaved
                        # groups lose earlier columns' start contributions).
                        for tl in range(TH):
                            t = th0 + tl
                            for ci in range(L1_CI):
                                nc.tensor.matmul(
                                    yps[:, t : t + 1],
                                    lhsT=wtts[ci][:, tl, :],
                                    rhs=x1t[:, ci, t : t + 1],
                                    start=(ci == 0),
                                    stop=(ci == L1_CI - 1),
                                )
                    m1 = xpool.tile([L1_PE, P], F32, tag="m1", name=f"m1_{g}")
                    nc.scalar.copy(m1, yps)
                    pst2 = ppool.tile([P, L1_PE], F32, tag="pst2", name=f"pst2_{g}")
                    nc.tensor.transpose(pst2, m1, ident[:L1_PE, :L1_PE])
                    nc.scalar.copy(y[:, L1_DVE:O], pst2)

                for c0 in range(0, o_dve, OCHUNK[li]):
                    oc = min(OCHUNK[li], o_dve - c0)
                    wt = wpool.tile([P, oc, I], F32, tag="w", name=f"w_{g}_{li}_{c0}")
                    nc.sync.dma_start(
                        out=wt, in_=W_d[li][n0 : n0 + P, c0 : c0 + oc, :]
                    )
                    if I <= 64:
                        # Layer 0: I is tiny, so per-o fused ops are
                        # overhead-dominated. Instead: one in-place batched
                        # multiply (x broadcast across the o dim via a
                        # step-0 AP) + one segmented 3D reduce.
                        x_b = bass.AP(
                            tensor=x.tensor,
                            offset=x.offset,
                            ap=[x.ap[0], [0, oc], x.ap[1]],
                        )
                        nc.vector.tensor_mul(wt, wt, x_b)
                        nc.vector.reduce_sum(
                            out=y[:, c0 : c0 + oc],
                            in_=wt,
                            axis=mybir.AxisListType.X,
                        )
                    else:
                        scr = spool.tile(
                            [P, I], F32, tag="scr", name=f"scr_{g}_{li}_{c0}"
                        )
                        for o in range(oc):
                            # accum_out = sum_i W[o,i]*x[i]  (custom DVE
                            # fused multiply-reduce; the ISA
                            # TENSOR_TENSOR_REDUCE opcode crashes TRN2
                            # hardware on this path)
                            nc.vector.affine_mul_reduce(
                                out=scr,
                                accum_out=y[:, c0 + o : c0 + o + 1],
                                in0=wt[:, o, :],
                                in1=x,
                                scale=1.0,
                                bias=0.0,
                            )
                nc.vector.tensor_add(y, y, bt)
                if li < len(DIMS) - 1:
                    # ELU(y) = exp(min(y,0)) + max(y,0) - 1
                    e = spool.tile([P, O], F32, tag="elu", name=f"e_{g}_{li}")
                    nc.vector.tensor_scalar_min(e, y, 0.0)
                    nc.scalar.activation(e, e, AF.Exp)
                    xn = xpool.tile([P, O], F32, tag="x", name=f"x_{g}_{li}")
                    nc.vector.scalar_tensor_tensor(
                        out=xn, in0=y, scalar=0.0, in1=e,
                        op0=ALU.max, op1=ALU.add,
                    )
                    nc.vector.tensor_scalar_add(xn, xn, -1.0)
                    return xn
                yt = xpool.tile([P, O], F32, tag="yt", name=f"yt_{g}")
                nc.scalar.activation(yt, y, AF.Tanh)
                nc.scalar.dma_start(out=out_d[n0 : n0 + P, :], in_=yt)
                return None

        # Staggered two-group pipeline: group 1 runs one layer behind group
        # 0 so DVE-heavy L0 work overlaps the other group's DMA-heavy L1
        # phase (and the PE matvec phase always has DVE work available).
        for g in range(G):
            xg = emit_norm(g)
            for li in range(len(DIMS)):
                xg = emit_layer(g, li, xg)

    nc.compile()
    return nc


def _get_nc():
    if "nc" not in _cached:
        _cached["nc"] = _build_bass()
    return _cached["nc"]


def _pack_core_inputs(full, c):
    """Shard + lay out one core's inputs (including the transposed W1 slice)."""
    sl = slice(c * NPC, (c + 1) * NPC)
    m = {
        k: np.ascontiguousarray(np.asarray(v)[sl])
        for k, v in full.items()
        if k != "W1"
    }
    w1c = np.asarray(full["W1"])[sl]  # [NPC, 256, 512]
    m["W1"] = np.ascontiguousarray(w1c[:, :L1_DVE, :])
    if USE_PE:
        w1b = w1c[:, L1_DVE:, :]  # [NPC, L1_PE, 512]
        # -> [g, ci, i_local, teacher, o]
        m["W1T"] = np.ascontiguousarray(
            w1b.reshape(G, P, L1_PE, L1_CI, P).transpose(0, 3, 4, 1, 2)
        )
    return m


def kernel(obs, mean, std, W0, b0, W1, b1, W2, b2, W3, b3, _trace=False):
    nc = _get_nc()
    full = {
        "obs": obs, "mean": mean, "std": std,
        "W0": W0, "b0": b0, "W1": W1, "b1": b1,
        "W2": W2, "b2": b2, "W3": W3, "b3": b3,
    }
    in_maps = [_pack_core_inputs(full, c) for c in range(N_CORES)]
    res = run_bass_kernel_spmd(
        nc, in_maps, core_ids=list(range(N_CORES)), trace=_trace
    )
    _cached["last_results"] = res
    out = np.concatenate([res.results[c]["out"] for c in range(N_CORES)], axis=0)
    return out



# revision 4
# speedup vs baseline: 1.7219x; 1.7219x over previous
"""Trainium2 Bass kernel for nn_BatchedTeacherPolicy.

2048 independent per-teacher MLPs (obs-norm -> 48->512->256->128->12,
ELU between layers, tanh at the end). Pure data parallel: 256 teachers
per NeuronCore across 8 cores, 2 groups of 128 teachers per core.

Strategy (v2):
- All weights cast to fp16 on the host (harness gate is rel_err < 2e-2;
  fp16 weights land ~1e-3) -> halves HBM traffic, the roofline for this
  memory-bound problem.
- L0 (48->512) on DVE, teacher-on-partition: fp16 tensor_tensor mul in
  2x_1p mode + segmented reduce.
- L1 (512->256) and L2 (256->128) on TensorE: per-teacher self-loading
  [128i x 128o] fp16 matmuls (FWL engages at 128 weight columns),
  accumulating each teacher's output column in PSUM across i-chunks.
  Weights are host-transposed to [group, ichunk, i, teacher, o] so each
  DMA is contiguous per partition and lhsT slices are step-1.
- Activations + per-teacher biases applied on the transposed [o, t]
  tiles (bias tiles host-transposed too).
- L3 (128->12) back on DVE (12 columns would waste PE weight loads)
  after one PE transpose; tanh on ACT; contiguous output DMA.
"""

from contextlib import ExitStack

import numpy as np

import concourse.bass as bass
import concourse.bacc as bacc
import concourse.tile as tile
from concourse import mybir
from concourse.bass_utils import run_bass_kernel_spmd

N, OBS = 2048, 48
DIMS = [(512, 48), (256, 512), (128, 256), (12, 128)]  # (out, in) per layer
N_CORES = 8
NPC = N // N_CORES  # teachers per core
P = 128             # partitions = teachers per group
G = NPC // P        # groups per core
TB = 16             # teachers per PE weight DMA chunk
TC = P // TB        # t-chunks per group

O1, I1 = DIMS[1]
O2, I2 = DIMS[2]
O3, I3 = DIMS[3]
IC1 = I1 // P       # 4 contraction chunks for L1
OC1 = O1 // P       # 2 output chunks for L1
IC2 = I2 // P       # 2 contraction chunks for L2

F32 = mybir.dt.float32
F16 = mybir.dt.float16
AF = mybir.ActivationFunctionType
ALU = mybir.AluOpType

_cached = {}


def _build_bass():
    nc = bacc.Bacc(trn_type="TRN2", target_bir_lowering=False)

    obs_d = nc.dram_tensor("obs", [NPC, OBS], F32, kind="ExternalInput")
    mean_d = nc.dram_tensor("mean", [NPC, OBS], F32, kind="ExternalInput")
    std_d = nc.dram_tensor("std", [NPC, OBS], F32, kind="ExternalInput")
    w0_d = nc.dram_tensor("W0", [NPC, 512, 48], F16, kind="ExternalInput")
    b0_d = nc.dram_tensor("b0", [NPC, 512], F32, kind="ExternalInput")
    w1t_d = nc.dram_tensor("W1T", [G, IC1, P, P, O1], F16, kind="ExternalInput")
    b1t_d = nc.dram_tensor("b1T", [G, O1, P], F32, kind="ExternalInput")
    w2t_d = nc.dram_tensor("W2T", [G, IC2, P, P, O2], F16, kind="ExternalInput")
    b2t_d = nc.dram_tensor("b2T", [G, O2, P], F32, kind="ExternalInput")
    w3_d = nc.dram_tensor("W3", [NPC, O3, I3], F16, kind="ExternalInput")
    b3_d = nc.dram_tensor("b3", [NPC, O3], F32, kind="ExternalInput")
    out_d = nc.dram_tensor("out", [NPC, O3], F32, kind="ExternalOutput")

    from concourse.masks import make_identity

    with ExitStack() as ctx:
        tc = ctx.enter_context(tile.TileContext(nc))
        wpool = ctx.enter_context(tc.tile_pool(name="wpool", bufs=2))
        xpool = ctx.enter_context(tc.tile_pool(name="xpool", bufs=2))
        spool = ctx.enter_context(tc.tile_pool(name="spool", bufs=2))
        bpool = ctx.enter_context(tc.tile_pool(name="bpool", bufs=2))
        ppool = ctx.enter_context(tc.tile_pool(name="ppool", bufs=2, space="PSUM"))
        ipool = ctx.enter_context(tc.tile_pool(name="ipool", bufs=1))

        ident = ipool.tile([P, P], F16)
        make_identity(nc, ident)

        def emit_norm(g):
            n0 = g * P
            # x0 = clip((obs - mean)/std, -5, 5), cast to fp16
            obs_t = spool.tile([P, OBS], F32, tag="nrm")
            nc.sync.dma_start(out=obs_t, in_=obs_d[n0 : n0 + P, :])
            mean_t = spool.tile([P, OBS], F32, tag="nrm")
            nc.sync.dma_start(out=mean_t, in_=mean_d[n0 : n0 + P, :])
            std_t = spool.tile([P, OBS], F32, tag="nrm")
            nc.sync.dma_start(out=std_t, in_=std_d[n0 : n0 + P, :])

            # Each DVE op may carry at most ONE new semaphore wait (TRN2
            # TT-struct limit), so feed multi-operand ops through
            # single-input ops that absorb the DMA waits first.
            nmean = spool.tile([P, OBS], F32, tag="nmean")
            nc.vector.tensor_scalar_mul(nmean, mean_t, -1.0)
            rstd = spool.tile([P, OBS], F32, tag="rstd")
            nc.vector.reciprocal(rstd, std_t)
            x = spool.tile([P, OBS], F32, tag="x0f", name=f"x0f_{g}")
            nc.vector.tensor_add(x, obs_t, nmean)
            nc.vector.tensor_mul(x, x, rstd)
            x_h = xpool.tile([P, OBS], F16, tag="x0h", name=f"x0h_{g}")
            nc.vector.tensor_scalar(
                out=x_h, in0=x, scalar1=-5.0, scalar2=5.0,
                op0=ALU.max, op1=ALU.min,
            )
            return x_h

        def emit_elu_tile(g, tag, ps_ap, bt, out_h):
            """out_h (fp16) = ELU(ps_ap + bt) for one [128, 128] transposed
            tile; ps_ap is PSUM fp32, bt a [128,128] f32 SBUF bias tile."""
            yb = spool.tile([P, P], F32, tag=f"yb", name=f"yb_{tag}_{g}")
            nc.vector.tensor_add(yb, ps_ap, bt)
            m_h = spool.tile([P, P], F16, tag="eluh", name=f"m_{tag}_{g}")
            nc.vector.tensor_scalar_min(m_h, yb, 0.0)
            e_h = spool.tile([P, P], F16, tag="eluh", name=f"e_{tag}_{g}")
            nc.scalar.activation(e_h, m_h, AF.Exp)
            # (max(yb,0) + e) - 1
            nc.vector.scalar_tensor_tensor(
                out=out_h, in0=yb, scalar=0.0, in1=e_h,
                op0=ALU.max, op1=ALU.add,
            )
            nc.vector.tensor_scalar_add(out_h, out_h, -1.0)

        def emit_group(g):
            n0 = g * P
            x_h = emit_norm(g)

            # ---- L0 on DVE: y0[t, o] = sum_i W0[t,o,i] x[t,i] ----
            y0 = spool.tile([P, 512], F32, tag="y0", name=f"y0_{g}")
            for c0 in range(0, 512, 128):
                wt = wpool.tile([P, 128, 48], F16, tag="w0", bufs=3,
                                name=f"w0_{g}_{c0}")
                nc.sync.dma_start(out=wt, in_=w0_d[n0 : n0 + P, c0 : c0 + 128, :])
                x_b = bass.AP(
                    tensor=x_h.tensor,
                    offset=x_h.offset,
                    ap=[x_h.ap[0], [0, 128], x_h.ap[1]],
                )
                nc.vector.tensor_mul(wt, wt, x_b)  # fp16 2x_1p
                nc.vector.reduce_sum(
                    out=y0[:, c0 : c0 + 128], in_=wt, axis=mybir.AxisListType.X
                )
            bt0 = bpool.tile([P, 512], F32, tag="b0", name=f"b0_{g}")
            nc.sync.dma_start(out=bt0, in_=b0_d[n0 : n0 + P, :])
            nc.vector.tensor_add(y0, y0, bt0)
            # ELU -> x1 fp16 [t, 512]
            m0 = spool.tile([P, 512], F16, tag="m0", name=f"m0_{g}")
            nc.vector.tensor_scalar_min(m0, y0, 0.0)
            e0 = spool.tile([P, 512], F16, tag="e0", name=f"e0_{g}")
            nc.scalar.activation(e0, m0, AF.Exp)
            x1_h = xpool.tile([P, 512], F16, tag="x1h", name=f"x1h_{g}")
            nc.vector.scalar_tensor_tensor(
                out=x1_h, in0=y0, scalar=0.0, in1=e0,
                op0=ALU.max, op1=ALU.add,
            )
            nc.vector.tensor_scalar_add(x1_h, x1_h, -1.0)

            # ---- transpose x1 -> x1t[ic] = [128 i, 128 t] fp16 ----
            x1t = []
            for ic in range(IC1):
                pst = ppool.tile([P, P], F16, tag="pst", name=f"pst_{g}_{ic}")
                nc.tensor.transpose(pst, x1_h[:, ic * P : (ic + 1) * P], ident)
                xt = xpool.tile([P, P], F16, tag="x1t", bufs=2 * IC1,
                                name=f"x1t_{g}_{ic}")
                nc.scalar.copy(xt, pst)
                x1t.append(xt)

            # ---- L1 on PE: yps[o, t] accumulated over 4 i-chunks ----
            yps = ppool.tile([P, OC1, P], F32, tag="yps", name=f"yps_{g}")
            for tcn in range(TC):
                t0 = tcn * TB
                w1s = []
                for ic in range(IC1):
                    w1 = wpool.tile([P, TB, O1], F16, tag="w1", bufs=2 * IC1,
                                    name=f"w1_{g}_{tcn}_{ic}")
                    nc.sync.dma_start(
                        out=w1, in_=w1t_d[g, ic, :, t0 : t0 + TB, :]
                    )
                    w1s.append(w1)
                for tl in range(TB):
                    t = t0 + tl
                    for oc in range(OC1):
                        for ic in range(IC1):
                            nc.tensor.matmul(
                                yps[:, oc, t : t + 1],
                                lhsT=w1s[ic][:, tl, oc * P : (oc + 1) * P],
                                rhs=x1t[ic][:, t : t + 1],
                                start=(ic == 0),
                                stop=(ic == IC1 - 1),
                            )

            # ---- L1 bias + ELU on transposed tiles -> x2t[oc] ----
            x2t = []
            for oc in range(OC1):
                bt1 = bpool.tile([P, P], F32, tag="b1t", name=f"b1t_{g}_{oc}")
                nc.sync.dma_start(out=bt1, in_=b1t_d[g, oc * P : (oc + 1) * P, :])
                xt = xpool.tile([P, P], F16, tag="x2t", bufs=2 * OC1,
                                name=f"x2t_{g}_{oc}")
                emit_elu_tile(g, f"l1_{oc}", yps[:, oc, :], bt1, xt)
                x2t.append(xt)

            # ---- L2 on PE ----
            y2ps = ppool.tile([P, P], F32, tag="y2ps", name=f"y2ps_{g}")
            for tcn in range(TC):
                t0 = tcn * TB
                w2s = []
                for ic in range(IC2):
                    w2 = wpool.tile([P, TB, O2], F16, tag="w2", bufs=2 * IC2,
                                    name=f"w2_{g}_{tcn}_{ic}")
                    nc.sync.dma_start(
                        out=w2, in_=w2t_d[g, ic, :, t0 : t0 + TB, :]
                    )
                    w2s.append(w2)
                for tl in range(TB):
                    t = t0 + tl
                    for ic in range(IC2):
                        nc.tensor.matmul(
                            y2ps[:, t : t + 1],
                            lhsT=w2s[ic][:, tl, :],
                            rhs=x2t[ic][:, t : t + 1],
                            start=(ic == 0),
                            stop=(ic == IC2 - 1),
                        )

            # ---- L2 bias + ELU -> x3t fp16 [128 o2, 128 t] ----
            bt2 = bpool.tile([P, P], F32, tag="b2t", name=f"b2t_{g}")
            nc.sync.dma_start(out=bt2, in_=b2t_d[g, :, :])
            x3t = xpool.tile([P, P], F16, tag="x3t", name=f"x3t_{g}")
            emit_elu_tile(g, "l2", y2ps, bt2, x3t)

            # ---- transpose back -> x3 [t, i] ----
            pst2 = ppool.tile([P, P], F16, tag="pst2", name=f"pst2_{g}")
            nc.tensor.transpose(pst2, x3t, ident)
            x3_h = xpool.tile([P, P], F16, tag="x3h", name=f"x3h_{g}")
            nc.scalar.copy(x3_h, pst2)

            # ---- L3 on DVE (12 outputs) + tanh ----
            w3t = wpool.tile([P, O3, I3], F16, tag="w3", name=f"w3_{g}")
            nc.sync.dma_start(out=w3t, in_=w3_d[n0 : n0 + P, :, :])
            y3 = spool.tile([P, O3], F32, tag="y3", name=f"y3_{g}")
            scr = spool.tile([P, I3], F16, tag="scr", name=f"scr_{g}")
            for o in range(O3):
                nc.vector.affine_mul_reduce(
                    out=scr,
                    accum_out=y3[:, o : o + 1],
                    in0=w3t[:, o, :],
                    in1=x3_h,
                    scale=1.0,
                    bias=0.0,
                )
            bt3 = bpool.tile([P, O3], F32, tag="b3", name=f"b3_{g}")
            nc.sync.dma_start(out=bt3, in_=b3_d[n0 : n0 + P, :])
            nc.vector.tensor_add(y3, y3, bt3)
            yt = spool.tile([P, O3], F32, tag="yt", name=f"yt_{g}")
            nc.scalar.activation(yt, y3, AF.Tanh)
            nc.scalar.dma_start(out=out_d[n0 : n0 + P, :], in_=yt)

        for g in range(G):
            emit_group(g)

    nc.compile()
    return nc


def _get_nc():
    if "nc" not in _cached:
        _cached["nc"] = _build_bass()
    return _cached["nc"]


def _pack_core_inputs(full, c):
    """Shard + lay out one core's inputs (fp16 weights, PE-transposed W1/W2)."""
    sl = slice(c * NPC, (c + 1) * NPC)
    f16 = np.float16
    m = {
        "obs": np.ascontiguousarray(full["obs"][sl]),
        "mean": np.ascontiguousarray(full["mean"][sl]),
        "std": np.ascontiguousarray(full["std"][sl]),
        "W0": np.ascontiguousarray(full["W0"][sl].astype(f16)),
        "b0": np.ascontiguousarray(full["b0"][sl]),
        "W3": np.ascontiguousarray(full["W3"][sl].astype(f16)),
        "b3": np.ascontiguousarray(full["b3"][sl]),
    }
    # W1T[g, ic, i, t, o] = W1[g*128+t, o, ic*128+i]
    w1c = full["W1"][sl].astype(f16)  # [NPC, 256, 512]
    m["W1T"] = np.ascontiguousarray(
        w1c.reshape(G, P, O1, IC1, P).transpose(0, 3, 4, 1, 2)
    )
    m["b1T"] = np.ascontiguousarray(
        full["b1"][sl].reshape(G, P, O1).transpose(0, 2, 1)
    )
    # W2T[g, ic, i, t, o] = W2[g*128+t, o, ic*128+i]
    w2c = full["W2"][sl].astype(f16)  # [NPC, 128, 256]
    m["W2T"] = np.ascontiguousarray(
        w2c.reshape(G, P, O2, IC2, P).transpose(0, 3, 4, 1, 2)
    )
    m["b2T"] = np.ascontiguousarray(
        full["b2"][sl].reshape(G, P, O2).transpose(0, 2, 1)
    )
    return m


def kernel(obs, mean, std, W0, b0, W1, b1, W2, b2, W3, b3, _trace=False):
    nc = _get_nc()
    full = {
        "obs": np.asarray(obs), "mean": np.asarray(mean), "std": np.asarray(std),
        "W0": np.asarray(W0), "b0": np.asarray(b0),
        "W1": np.asarray(W1), "b1": np.asarray(b1),
        "W2": np.asarray(W2), "b2": np.asarray(b2),
        "W3": np.asarray(W3), "b3": np.asarray(b3),
    }
    in_maps = [_pack_core_inputs(full, c) for c in range(N_CORES)]
    res = run_bass_kernel_spmd(
        nc, in_maps, core_ids=list(range(N_CORES)), trace=_trace
    )
    _cached["last_results"] = res
    out = np.concatenate([res.results[c]["out"] for c in range(N_CORES)], axis=0)
    return out
